# revision 19
# baseline (speedup 1.0000x reference)
"""Graphormer multi-head attention on 8 Trainium2 cores.

Sharding: 2 cores per batch element (B=4), each core handling 8 of 16 heads
(tensor-parallel within the batch). Per core:
  - QKV projections for its 512 local feature columns (transposed layouts)
  - scoresT[s,t] = K_h Q_h^T per head (K=64 contraction on PE). Heads are
    processed in parity PAIRS: even head occupies PE rows 0-63, odd head
    rows 64-127 (tile_position row-tiling) so adjacent matmuls execute
    concurrently on disjoint row groups -> 2x QK throughput.
  - p = exp(scoresT) * expbT on ACT (exp) + DVE/GPSIMD (mul). expbT =
    exp(attn_mask + edge_bias).T from host; |scores| < ~8 so no
    max-subtraction needed.
  - PV with a ones-column appended to V -> row 64 of PSUM = softmax denom.
    PV runs as dense per-head bursts reading SBUF-buffered pT chunks, so
    PV PSUM tiles live only ~3.4us (2 banks).
  - normalize via reciprocal + partition-broadcast; out-project in two
    fc halves (yA = heads 0-3 mid-kernel, yB = heads 4-7 at the tail with
    PSUM drains on the then-idle scalar engine). Host sums the partials.
All matmuls bf16 with fp32 PSUM accumulation. DMAs are need-ordered and
fine-grained so the PE starts at ~3.5us and HAM stays warm.
"""
import sys

sys.path.insert(0, '/opt/trn_rl_repo')

import ml_dtypes
import numpy as np

import concourse.bass as bass
import concourse.mybir as mybir
import concourse.tile as tile
from concourse import bacc, library_config
from concourse.bass_utils import run_bass_kernel_spmd

DT = mybir.dt

B, T, S, E, H = 4, 1024, 1024, 1024, 16
D = E // H          # 64
HL = 8              # heads per core
F = HL * D          # 512 local features
N_CORES = 8

MMDT = DT.bfloat16
NP_MMDT = ml_dtypes.bfloat16
EXP = mybir.ActivationFunctionType.Exp


def _build_program():
    nc = bacc.Bacc()

    # packed host layouts (2-8KB contiguous DMA lines):
    #   xq/xk/xv: [128, th 2, ec 8, 512];  wq/wk: [128, fc 4, ec 8, 128]
    #   wv: [128, ec 8, f 512]
    xqT = nc.dram_tensor("xqT", [128, 2 * 8 * 512], MMDT, kind="ExternalInput")
    xkT = nc.dram_tensor("xkT", [128, 2 * 8 * 512], MMDT, kind="ExternalInput")
    xvT = nc.dram_tensor("xvT", [128, 2 * 8 * 512], MMDT, kind="ExternalInput")
    wqT = nc.dram_tensor("wqT", [128, 4 * 8 * 128], MMDT, kind="ExternalInput")
    wkT = nc.dram_tensor("wkT", [128, 4 * 8 * 128], MMDT, kind="ExternalInput")
    wvT = nc.dram_tensor("wvT", [128, 8 * 512], MMDT, kind="ExternalInput")
    woT = nc.dram_tensor("woT", [128, 4 * E], MMDT, kind="ExternalInput")
    bq = nc.dram_tensor("bq", [128, 4], DT.float32, kind="ExternalInput")
    bk = nc.dram_tensor("bk", [128, 4], DT.float32, kind="ExternalInput")
    bv = nc.dram_tensor("bv", [1, F], DT.float32, kind="ExternalInput")
    expbT = nc.dram_tensor("expbT", [S, T], MMDT, kind="ExternalInput")
    yA = nc.dram_tensor("yA", [E, T], DT.float32, kind="ExternalOutput")
    yB = nc.dram_tensor("yB", [E, T], DT.float32, kind="ExternalOutput")

    def rr(t, expr, **kw):
        return t.rearrange(expr, **kw)

    with tile.TileContext(nc) as tc:
        with tc.tile_pool(name="persist", bufs=1) as pp, \
             tc.tile_pool(name="xin", bufs=1) as xp, \
             tc.tile_pool(name="pT", bufs=11) as pTp, \
             tc.tile_pool(name="et", bufs=2) as etp, \
             tc.tile_pool(name="nrm", bufs=2) as nrmp, \
             tc.tile_pool(name="ysb", bufs=2) as ysp, \
             tc.tile_pool(name="sc", bufs=2, space="PSUM") as ps_s, \
             tc.tile_pool(name="pv", bufs=2, space="PSUM") as ps_pv, \
             tc.tile_pool(name="acc", bufs=2, space="PSUM") as ps_a:

            # ---- persistent SBUF tiles ----
            wq_sb = pp.tile([128, 4, 8, 128], MMDT, tag="wq")
            wk_sb = pp.tile([128, 4, 8, 128], MMDT, tag="wk")
            wv_sb = pp.tile([128, 8, F], MMDT, tag="wv")
            wo_sb = pp.tile([128, 4, E], MMDT, tag="wo")
            xq_sb = xp.tile([128, 2, 8, 512], MMDT, tag="xq")
            xk_sb = xp.tile([128, 2, 8, 512], MMDT, tag="xk")
            xv_sb = xp.tile([128, 2, 8, 512], MMDT, tag="xv")
            qT_sb = pp.tile([128, 4, T], MMDT, tag="qT")
            kT_sb = pp.tile([128, 4, S], MMDT, tag="kT")
            v_sb = pp.tile([128, 8, HL * 128], MMDT, tag="v")
            expb_sb = pp.tile([128, 8, T], MMDT, tag="expb")
            oT_sb = pp.tile([128, 4, T], MMDT, tag="oT")
            bq_sb = pp.tile([128, 4], DT.float32, tag="bq")
            bk_sb = pp.tile([128, 4], DT.float32, tag="bk")
            bv_row = pp.tile([1, F], DT.float32, tag="bvrow")
            bv_bc = pp.tile([128, F], DT.float32, tag="bvbc")
            tiny_in = pp.tile([1, 8], DT.float32, tag="tiny_i")
            tiny_out = pp.tile([1, 8], DT.float32, tag="tiny_o")

            # ---- ACT exp-table preload (runs during initial DMA wait) ----
            nc.vector.memset(tiny_in[:], 0.0)
            nc.scalar.activation(tiny_out[:], tiny_in[:], EXP)

            # ---- DMA triggers, need-ordered on the sync queue ----
            # packed layouts -> contiguous 2-8KB lines per partition.
            # expb chunks are just-in-time interleaved so the early HBM
            # bandwidth goes to the projection inputs.
            def ld_w(dst, src, fc0, fc1):
                nc.sync.dma_start(
                    dst[:, fc0:fc1],
                    rr(src[:, fc0 * 1024:fc1 * 1024], "p (c e k) -> p c e k",
                       c=fc1 - fc0, e=8))

            def ld_x(dst, src, th):
                nc.sync.dma_start(
                    dst[:, th],
                    rr(src[:, th * 4096:(th + 1) * 4096], "p (e k) -> p e k",
                       e=8))

            def ld_expb(sc):
                nc.sync.dma_start(
                    expb_sb[:, sc, :], expbT[sc * 128:(sc + 1) * 128, :])

            # small tensors + lib pin first on the gpsimd queue
            nc.gpsimd.dma_start(bq_sb[:], bq[:])
            nc.gpsimd.dma_start(bk_sb[:], bk[:])
            nc.gpsimd.dma_start(bv_row[:], bv[:])
            nc.gpsimd.load_library(library_config.proxy)
            nc.gpsimd.partition_broadcast(bv_bc[:], bv_row[:])

            # critical first loads split across the sync + scalar queues so
            # two DGE streams run in parallel
            ld_w(wq_sb, wqT, 0, 1)
            nc.scalar.dma_start(
                xq_sb[:, 0], rr(xqT[:, 0:4096], "p (e k) -> p e k", e=8))
            nc.sync.dma_start(
                xk_sb[:, 0], rr(xkT[:, 0:4096], "p (e k) -> p e k", e=8))
            nc.scalar.dma_start(
                wk_sb[:, 0:1], rr(wkT[:, 0:1024], "p (c e k) -> p c e k",
                                  c=1, e=8))
            ld_x(xq_sb, xqT, 1)
            nc.scalar.dma_start(
                xk_sb[:, 1], rr(xkT[:, 4096:8192], "p (e k) -> p e k", e=8))
            ld_expb(0)
            ld_expb(1)
            nc.sync.dma_start(wv_sb[:], rr(wvT[:], "p (e k) -> p e k", e=8))
            nc.sync.dma_start(xv_sb[:], rr(xvT[:], "p (a e k) -> p a e k",
                                           a=2, e=8))
            ld_expb(2)
            ld_w(wq_sb, wqT, 1, 4)
            ld_w(wk_sb, wkT, 1, 4)
            ld_expb(3)
            ld_expb(4)
            ld_expb(5)
            nc.sync.dma_start(wo_sb[:], rr(woT[:], "p (c e) -> p c e", c=4))
            ld_expb(6)
            ld_expb(7)

            # ---- emitters ----
            def proj_fc_th(x_sb, w_sb, b_sb, dstT, fc, th):
                acc = ps_a.tile([128, 512], DT.float32, tag="acc")
                for ec in range(8):
                    nc.tensor.matmul(
                        acc[:],
                        w_sb[:, fc, ec, :],
                        x_sb[:, th, ec, :],
                        start=(ec == 0), stop=(ec == 7),
                    )
                dst = dstT[:, fc, th * 512:(th + 1) * 512]
                if fc == 0:
                    # pre-attention: ACT is idle, use it for the drain
                    nc.scalar.add(dst, acc[:], b_sb[:, fc:fc + 1])
                else:
                    nc.vector.tensor_scalar_add(dst, acc[:],
                                                b_sb[:, fc:fc + 1])

            def vproj_sc(sc):
                acc = ps_a.tile([128, 512], DT.float32, tag="acc")
                th, o = sc // 4, (sc % 4) * 128
                for ec in range(8):
                    nc.tensor.matmul(
                        acc[:],
                        xv_sb[:, th, ec, o:o + 128],
                        wv_sb[:, ec, :],
                        start=(ec == 0), stop=(ec == 7),
                    )
                vv = v_sb[:, sc, :].rearrange("p (h c) -> p h c", c=128)
                nc.vector.tensor_add(
                    vv[:, :, 0:64],
                    acc[:].rearrange("p (h d) -> p h d", d=64),
                    bv_bc[:].rearrange("p (h d) -> p h d", d=64),
                )
                nc.vector.memset(vv[:, :, 64:128], 1.0)

            pT_tiles = {}

            def qk_sc(c, sc):
                # even head on PE rows 0-63, odd on rows 64-127: the four
                # matmuls execute pairwise-concurrently via row tiling.
                pTt = pTp.tile([128, 2 * T], MMDT, tag="pT")
                pT_tiles[(c, sc)] = pTt
                et = etp.tile([128, 2, T], MMDT, tag="et")
                for par in range(2):
                    po = par * 64
                    sps = ps_s.tile([128, T], DT.float32, tag="sc")
                    for th in range(2):
                        nc.tensor.matmul(
                            sps[:, th * 512:(th + 1) * 512],
                            kT_sb[po:po + 64, c, sc * 128:(sc + 1) * 128],
                            qT_sb[po:po + 64, c, th * 512:(th + 1) * 512],
                            start=True, stop=True,
                        )
                    nc.scalar.activation(et[:, par, :], sps[:], EXP)
                # one 2048-wide multiply for both parities; expb repeats via
                # a zero-stride broadcast AP (HW-validated)
                eng = nc.gpsimd if sc in (0, 3, 6) else nc.vector
                eng.tensor_mul(
                    pTt[:].rearrange("p (a b) -> p a b", a=2),
                    et[:],
                    expb_sb[:, sc, :].unsqueeze(1).broadcast_to([128, 2, T]))

            def pv_mms(c, par, th, j, ops):
                # accumulation matmuls for s-chunks 2j, 2j+1 of head pair c,
                # parity par, t-half th
                h = 2 * c + par
                off = par * T + th * 512
                for m in (2 * j, 2 * j + 1):
                    nc.tensor.matmul(
                        ops[:],
                        v_sb[:, m, h * 128:(h + 1) * 128],
                        pT_tiles[(c, m)][:, off:off + 512],
                        start=(m == 0), stop=(m == 7),
                    )

            def pv_norm(c, par, th, ops):
                denb = nrmp.tile([64, 512], DT.float32, tag="denb")
                nc.vector.tensor_copy(denb[:], ops[64:128, :])
                rb = nrmp.tile([64, 512], DT.float32, tag="rb")
                nc.vector.reciprocal_approx_fast(out=rb[:], in_=denb[:])
                po = par * 64
                nc.vector.tensor_mul(
                    oT_sb[po:po + 64, c, th * 512:(th + 1) * 512],
                    ops[0:64, :], rb[:])

            def pv_slot(c, sc, pvst):
                # one slot of the software-pipelined PV for pair c: t-half
                # th = sc//4 accumulates s-chunks 2j,2j+1 (j = sc%4) for both
                # parities; norm at j == 3. PSUM cost: 2 banks.
                th, j = sc // 4, sc % 4
                if j == 0:
                    ops_e = ps_pv.tile([128, 512], DT.float32, tag="pv")
                    ops_o = ps_pv.tile([128, 512], DT.float32, tag="pv")
                    pvst[0], pvst[1] = ops_e, ops_o
                for par in range(2):
                    pv_mms(c, par, th, j, pvst[par])
                if j == 3:
                    for par in range(2):
                        pv_norm(c, par, th, pvst[par])

            def y_chunk(half, ec8, th):
                # half 0: fc 0,1 (heads 0-3) -> yA, DVE drains
                # half 1: fc 2,3 (heads 4-7) -> yB, ACT drains (idle at tail)
                fcs = (0, 1) if half == 0 else (2, 3)
                yps = ps_a.tile([128, 512], DT.float32, tag="acc")
                for i, fc in enumerate(fcs):
                    nc.tensor.matmul(
                        yps[:],
                        wo_sb[:, fc, ec8 * 128:(ec8 + 1) * 128],
                        oT_sb[:, fc, th * 512:(th + 1) * 512],
                        start=(i == 0), stop=(i == 1),
                    )
                ysb = ysp.tile([128, 512], DT.float32, tag="ysb")
                if half == 0 or ec8 % 2 == 1:
                    nc.vector.tensor_copy(ysb[:], yps[:])
                else:
                    nc.scalar.copy(ysb[:], yps[:])
                dst = yA if half == 0 else yB
                nc.sync.dma_start(
                    dst[ec8 * 128:(ec8 + 1) * 128, th * 512:(th + 1) * 512],
                    ysb[:])

            # ---- phase 1: q/k projections for head-pair 0 ----
            for th in range(2):
                proj_fc_th(xq_sb, wq_sb, bq_sb, qT_sb, 0, th)
            for th in range(2):
                proj_fc_th(xk_sb, wk_sb, bk_sb, kT_sb, 0, th)

            # ---- phase 2: attention windows with PE filler work ----
            def mk_proj(x, w, b, d, fc, th):
                return lambda: proj_fc_th(x, w, b, d, fc, th)

            windows = {
                0: [(lambda sc=sc: vproj_sc(sc)) for sc in range(6)]
                   + [mk_proj(xq_sb, wq_sb, bq_sb, qT_sb, 1, th) for th in range(2)]
                   + [mk_proj(xk_sb, wk_sb, bk_sb, kT_sb, 1, th) for th in range(2)],
                1: [(lambda sc=sc: vproj_sc(sc)) for sc in range(6, 8)]
                   + [mk_proj(xq_sb, wq_sb, bq_sb, qT_sb, 2, th) for th in range(2)]
                   + [mk_proj(xk_sb, wk_sb, bk_sb, kT_sb, 2, th) for th in range(2)],
                2: [mk_proj(xq_sb, wq_sb, bq_sb, qT_sb, 3, th) for th in range(2)]
                   + [mk_proj(xk_sb, wk_sb, bk_sb, kT_sb, 3, th) for th in range(2)],
                3: [],
            }

            for c in range(4):
                fillers = windows[c]
                n = len(fillers)
                pvst = {}
                for sc in range(8):
                    qk_sc(c, sc)
                    if c >= 1:
                        pv_slot(c - 1, sc, pvst)
                    if c == 3:
                        # yA: needs pair 0/1 norms, all done by end of w2
                        y_chunk(0, sc, 0)
                        y_chunk(0, sc, 1)
                    for i in range(n * sc // 8, n * (sc + 1) // 8):
                        fillers[i]()

            # ---- phase 3: tail — PV of pair 3 + yB ----
            pvst = {}
            for sc in range(8):
                pv_slot(3, sc, pvst)
                if sc >= 4:
                    # pair-3 th0 norms land at slot 3
                    for e in (2 * (sc - 4), 2 * (sc - 4) + 1):
                        y_chunk(1, e, 0)
            for e in range(8):
                y_chunk(1, e, 1)

    nc.compile()
    return nc


_NC_CACHE = []


def kernel(query, key_, value, edge_bias, attn_mask, key_padding_mask,
           Wq, bq, Wk, bk, Wv, bv, Wo, bo):
    if not _NC_CACHE:
        _NC_CACHE.append(_build_program())
    nc = _NC_CACHE[0]

    scale = np.float32(D ** -0.5)
    q32, k32, v32 = (np.asarray(a, np.float32) for a in (query, key_, value))
    WqT = (np.asarray(Wq, np.float32).T * scale).astype(NP_MMDT)
    WkT = np.asarray(Wk, np.float32).T.astype(NP_MMDT)
    WvT = np.asarray(Wv, np.float32).T.astype(NP_MMDT)
    WoT = np.asarray(Wo, np.float32).T
    bq_s = (np.asarray(bq, np.float32) * scale)
    kpm_add = np.where(np.asarray(key_padding_mask), np.float32(-1e30),
                       np.float32(0.0))  # [B, S]
    mask32 = np.asarray(attn_mask, np.float32)

    def pack_x(xT):
        # [E, T] -> [128, th 2, ec 8, 512] flattened
        return np.ascontiguousarray(
            xT.reshape(8, 128, 2, 512).transpose(1, 2, 0, 3)
            .reshape(128, 8192))

    def pack_w(wT):
        # [E, F] -> [128, fc 4, ec 8, 128] flattened
        return np.ascontiguousarray(
            wT.reshape(8, 128, 4, 128).transpose(1, 2, 0, 3)
            .reshape(128, 4096))

    def pack_wv(wT):
        # [E, F] -> [128, ec 8, 512] flattened
        return np.ascontiguousarray(
            wT.reshape(8, 128, 512).transpose(1, 0, 2).reshape(128, 4096))

    in_maps = []
    for c in range(N_CORES):
        b, g = divmod(c, 2)
        cols = slice(g * F, (g + 1) * F)
        bias_sb = (mask32 + np.asarray(edge_bias[b], np.float32)
                   + kpm_add[b][None, :])  # [T, S]
        in_maps.append({
            "xqT": pack_x(q32[b].T.astype(NP_MMDT)),
            "xkT": pack_x(k32[b].T.astype(NP_MMDT)),
            "xvT": pack_x(v32[b].T.astype(NP_MMDT)),
            "wqT": pack_w(WqT[:, cols]),
            "wkT": pack_w(WkT[:, cols]),
            "wvT": pack_wv(WvT[:, cols]),
            "woT": np.ascontiguousarray(
                WoT[cols, :].astype(NP_MMDT).reshape(4, 128, E)
                .transpose(1, 0, 2).reshape(128, 4 * E)),
            "bq": np.ascontiguousarray(bq_s[cols].reshape(4, 128).T),
            "bk": np.ascontiguousarray(np.asarray(bk, np.float32)[cols]
                                       .reshape(4, 128).T),
            "bv": np.asarray(bv, np.float32)[cols].reshape(1, F),
            "expbT": np.exp(bias_sb.T).astype(NP_MMDT),
        })

    res = run_bass_kernel_spmd(nc, in_maps, list(range(N_CORES)))

    out = np.empty((B, T, E), np.float32)
    bo32 = np.asarray(bo, np.float32)
    for b in range(B):
        r0, r1 = res.results[2 * b], res.results[2 * b + 1]
        acc = r0["yA"] + r0["yB"] + r1["yA"] + r1["yB"]
        out[b] = acc.T + bo32[None, :]
    return out


# revision 20
# speedup vs baseline: 1.0853x; 1.0853x over previous
"""Graphormer multi-head attention on 8 Trainium2 cores.

Sharding: 2 cores per batch element (B=4), each core handling 8 of 16 heads
(tensor-parallel within the batch). Per core:
  - QKV projections for its 512 local feature columns (transposed layouts)
  - scoresT[s,t] = K_h Q_h^T per head (K=64 contraction on PE). Heads are
    processed in parity PAIRS: even head occupies PE rows 0-63, odd head
    rows 64-127 (tile_position row-tiling) so adjacent matmuls execute
    concurrently on disjoint row groups -> 2x QK throughput.
  - p = exp(scoresT) * expbT on ACT (exp) + DVE/GPSIMD (mul). expbT =
    exp(attn_mask + edge_bias).T from host; |scores| < ~8 so no
    max-subtraction needed.
  - PV with a ones-column appended to V -> row 64 of PSUM = softmax denom.
    PV runs as dense per-head bursts reading SBUF-buffered pT chunks, so
    PV PSUM tiles live only ~3.4us (2 banks).
  - normalize via reciprocal + partition-broadcast; out-project in two
    fc halves (yA = heads 0-3 mid-kernel, yB = heads 4-7 at the tail with
    PSUM drains on the then-idle scalar engine). Host sums the partials.
All matmuls bf16 with fp32 PSUM accumulation. DMAs are need-ordered and
fine-grained so the PE starts at ~3.5us and HAM stays warm.
"""
import sys

sys.path.insert(0, '/opt/trn_rl_repo')

import ml_dtypes
import numpy as np

import concourse.bass as bass
import concourse.mybir as mybir
import concourse.tile as tile
from concourse import bacc, library_config
from concourse.bass_utils import run_bass_kernel_spmd

DT = mybir.dt

B, T, S, E, H = 4, 1024, 1024, 1024, 16
D = E // H          # 64
HL = 8              # heads per core
F = HL * D          # 512 local features
N_CORES = 8

MMDT = DT.bfloat16
NP_MMDT = ml_dtypes.bfloat16
EXP = mybir.ActivationFunctionType.Exp


def _build_program():
    nc = bacc.Bacc()

    # packed host layouts (2-8KB contiguous DMA lines):
    #   xq/xk/xv: [128, th 2, ec 8, 512];  wq/wk: [128, fc 4, ec 8, 128]
    #   wv: [128, ec 8, f 512]
    xqT = nc.dram_tensor("xqT", [128, 2 * 8 * 512], MMDT, kind="ExternalInput")
    xkT = nc.dram_tensor("xkT", [128, 2 * 8 * 512], MMDT, kind="ExternalInput")
    xvT = nc.dram_tensor("xvT", [128, 2 * 8 * 512], MMDT, kind="ExternalInput")
    wqT = nc.dram_tensor("wqT", [128, 4 * 8 * 128], MMDT, kind="ExternalInput")
    wkT = nc.dram_tensor("wkT", [128, 4 * 8 * 128], MMDT, kind="ExternalInput")
    wvT = nc.dram_tensor("wvT", [128, 8 * 512], MMDT, kind="ExternalInput")
    woT = nc.dram_tensor("woT", [128, 4 * E], MMDT, kind="ExternalInput")
    bq = nc.dram_tensor("bq", [128, 4], DT.float32, kind="ExternalInput")
    bk = nc.dram_tensor("bk", [128, 4], DT.float32, kind="ExternalInput")
    bv = nc.dram_tensor("bv", [1, F], DT.float32, kind="ExternalInput")
    expbT = nc.dram_tensor("expbT", [S, T], MMDT, kind="ExternalInput")
    yA = nc.dram_tensor("yA", [E, T], DT.float32, kind="ExternalOutput")
    yB = nc.dram_tensor("yB", [E, T], DT.float32, kind="ExternalOutput")

    def rr(t, expr, **kw):
        return t.rearrange(expr, **kw)

    with tile.TileContext(nc) as tc:
        with tc.tile_pool(name="persist", bufs=1) as pp, \
             tc.tile_pool(name="xin", bufs=1) as xp, \
             tc.tile_pool(name="pT", bufs=12) as pTp, \
             tc.tile_pool(name="et", bufs=3) as etp, \
             tc.tile_pool(name="nrm", bufs=1) as nrmp, \
             tc.tile_pool(name="ysb", bufs=2) as ysp, \
             tc.tile_pool(name="sc", bufs=2, space="PSUM") as ps_s, \
             tc.tile_pool(name="pv", bufs=2, space="PSUM") as ps_pv, \
             tc.tile_pool(name="acc", bufs=2, space="PSUM") as ps_a:

            # ---- persistent SBUF tiles ----
            wq_sb = pp.tile([128, 4, 8, 128], MMDT, tag="wq")
            wk_sb = pp.tile([128, 4, 8, 128], MMDT, tag="wk")
            wv_sb = pp.tile([128, 8, F], MMDT, tag="wv")
            wo_sb = pp.tile([128, 4, E], MMDT, tag="wo")
            xq_sb = xp.tile([128, 2, 8, 512], MMDT, tag="xq")
            xk_sb = xp.tile([128, 2, 8, 512], MMDT, tag="xk")
            xv_sb = xp.tile([128, 2, 8, 512], MMDT, tag="xv")
            qT_sb = pp.tile([128, 4, T], MMDT, tag="qT")
            kT_sb = pp.tile([128, 4, S], MMDT, tag="kT")
            v_sb = pp.tile([128, 8, HL * 128], MMDT, tag="v")
            expb_sb = pp.tile([128, 8, T], MMDT, tag="expb")
            oT_sb = pp.tile([128, 4, T], MMDT, tag="oT")
            bq_sb = pp.tile([128, 4], DT.float32, tag="bq")
            bk_sb = pp.tile([128, 4], DT.float32, tag="bk")
            bv_row = pp.tile([1, F], DT.float32, tag="bvrow")
            bv_bc = pp.tile([128, F], DT.float32, tag="bvbc")
            tiny_in = pp.tile([1, 8], DT.float32, tag="tiny_i")
            tiny_out = pp.tile([1, 8], DT.float32, tag="tiny_o")

            # ---- ACT exp-table preload (runs during initial DMA wait) ----
            nc.vector.memset(tiny_in[:], 0.0)
            nc.scalar.activation(tiny_out[:], tiny_in[:], EXP)

            # ---- DMA triggers, need-ordered on the sync queue ----
            # packed layouts -> contiguous 2-8KB lines per partition.
            # expb chunks are just-in-time interleaved so the early HBM
            # bandwidth goes to the projection inputs.
            def ld_w(dst, src, fc0, fc1):
                nc.sync.dma_start(
                    dst[:, fc0:fc1],
                    rr(src[:, fc0 * 1024:fc1 * 1024], "p (c e k) -> p c e k",
                       c=fc1 - fc0, e=8))

            def ld_x(dst, src, th):
                nc.sync.dma_start(
                    dst[:, th],
                    rr(src[:, th * 4096:(th + 1) * 4096], "p (e k) -> p e k",
                       e=8))

            def ld_expb(sc):
                nc.sync.dma_start(
                    expb_sb[:, sc, :], expbT[sc * 128:(sc + 1) * 128, :])

            # small tensors + lib pin first on the gpsimd queue
            nc.gpsimd.dma_start(bq_sb[:], bq[:])
            nc.gpsimd.dma_start(bk_sb[:], bk[:])
            nc.gpsimd.dma_start(bv_row[:], bv[:])
            nc.gpsimd.load_library(library_config.proxy)
            nc.gpsimd.partition_broadcast(bv_bc[:], bv_row[:])

            # critical first loads split across the sync + scalar queues so
            # two DGE streams run in parallel
            ld_w(wq_sb, wqT, 0, 1)
            nc.scalar.dma_start(
                xq_sb[:, 0], rr(xqT[:, 0:4096], "p (e k) -> p e k", e=8))
            nc.sync.dma_start(
                xk_sb[:, 0], rr(xkT[:, 0:4096], "p (e k) -> p e k", e=8))
            nc.scalar.dma_start(
                wk_sb[:, 0:1], rr(wkT[:, 0:1024], "p (c e k) -> p c e k",
                                  c=1, e=8))
            ld_x(xq_sb, xqT, 1)
            nc.scalar.dma_start(
                xk_sb[:, 1], rr(xkT[:, 4096:8192], "p (e k) -> p e k", e=8))
            ld_expb(0)
            ld_expb(1)
            nc.sync.dma_start(wv_sb[:], rr(wvT[:], "p (e k) -> p e k", e=8))
            nc.sync.dma_start(xv_sb[:], rr(xvT[:], "p (a e k) -> p a e k",
                                           a=2, e=8))
            ld_expb(2)
            ld_w(wq_sb, wqT, 1, 4)
            ld_w(wk_sb, wkT, 1, 4)
            ld_expb(3)
            ld_expb(4)
            ld_expb(5)
            nc.sync.dma_start(wo_sb[:], rr(woT[:], "p (c e) -> p c e", c=4))
            ld_expb(6)
            ld_expb(7)

            # ---- emitters ----
            def proj_fc_th(x_sb, w_sb, b_sb, dstT, fc, th):
                acc = ps_a.tile([128, 512], DT.float32, tag="acc")
                for ec in range(8):
                    nc.tensor.matmul(
                        acc[:],
                        w_sb[:, fc, ec, :],
                        x_sb[:, th, ec, :],
                        start=(ec == 0), stop=(ec == 7),
                    )
                dst = dstT[:, fc, th * 512:(th + 1) * 512]
                if fc == 0:
                    # pre-attention: ACT is idle, use it for the drain
                    nc.scalar.add(dst, acc[:], b_sb[:, fc:fc + 1])
                else:
                    nc.vector.tensor_scalar_add(dst, acc[:],
                                                b_sb[:, fc:fc + 1])

            def vproj_sc(sc):
                acc = ps_a.tile([128, 512], DT.float32, tag="acc")
                th, o = sc // 4, (sc % 4) * 128
                for ec in range(8):
                    nc.tensor.matmul(
                        acc[:],
                        xv_sb[:, th, ec, o:o + 128],
                        wv_sb[:, ec, :],
                        start=(ec == 0), stop=(ec == 7),
                    )
                vv = v_sb[:, sc, :].rearrange("p (h c) -> p h c", c=128)
                nc.vector.tensor_add(
                    vv[:, :, 0:64],
                    acc[:].rearrange("p (h d) -> p h d", d=64),
                    bv_bc[:].rearrange("p (h d) -> p h d", d=64),
                )
                nc.vector.memset(vv[:, :, 64:128], 1.0)

            pT_tiles = {}

            def qk_sc(c, sc):
                # even head on PE rows 0-63, odd on rows 64-127: the four
                # matmuls execute pairwise-concurrently via row tiling.
                pTt = pTp.tile([128, 2 * T], MMDT, tag="pT")
                pT_tiles[(c, sc)] = pTt
                for par in range(2):
                    po = par * 64
                    sps = ps_s.tile([128, T], DT.float32, tag="sc")
                    for th in range(2):
                        nc.tensor.matmul(
                            sps[:, th * 512:(th + 1) * 512],
                            kT_sb[po:po + 64, c, sc * 128:(sc + 1) * 128],
                            qT_sb[po:po + 64, c, th * 512:(th + 1) * 512],
                            start=True, stop=True,
                        )
                    et = etp.tile([128, T], MMDT, tag="et")
                    nc.scalar.activation(et[:], sps[:], EXP)
                    eng = nc.gpsimd if par == 1 else nc.vector
                    eng.tensor_mul(pTt[:, par * T:(par + 1) * T], et[:],
                                   expb_sb[:, sc, :])

            def pv_mms(c, par, th, j, ops):
                # accumulation matmuls for s-chunks 2j, 2j+1 of head pair c,
                # parity par, t-half th
                h = 2 * c + par
                off = par * T + th * 512
                for m in (2 * j, 2 * j + 1):
                    nc.tensor.matmul(
                        ops[:],
                        v_sb[:, m, h * 128:(h + 1) * 128],
                        pT_tiles[(c, m)][:, off:off + 512],
                        start=(m == 0), stop=(m == 7),
                    )

            def pv_norm(c, par, th, ops):
                denb = nrmp.tile([64, 512], DT.float32, tag="denb")
                nc.vector.tensor_copy(denb[:], ops[64:128, :])
                rb = nrmp.tile([64, 512], DT.float32, tag="rb")
                nc.vector.reciprocal_approx_fast(out=rb[:], in_=denb[:])
                po = par * 64
                nc.vector.tensor_mul(
                    oT_sb[po:po + 64, c, th * 512:(th + 1) * 512],
                    ops[0:64, :], rb[:])

            def pv_slot(c, sc, pvst):
                # one slot of the software-pipelined PV for pair c: t-half
                # th = sc//4 accumulates s-chunks 2j,2j+1 (j = sc%4) for both
                # parities; norm at j == 3. PSUM cost: 2 banks.
                th, j = sc // 4, sc % 4
                if j == 0:
                    ops_e = ps_pv.tile([128, 512], DT.float32, tag="pv")
                    ops_o = ps_pv.tile([128, 512], DT.float32, tag="pv")
                    pvst[0], pvst[1] = ops_e, ops_o
                for par in range(2):
                    pv_mms(c, par, th, j, pvst[par])
                if j == 3:
                    for par in range(2):
                        pv_norm(c, par, th, pvst[par])

            def y_chunk(half, ec8, th):
                # half 0: fc 0,1 (heads 0-3) -> yA, DVE drains
                # half 1: fc 2,3 (heads 4-7) -> yB, ACT drains (idle at tail)
                fcs = (0, 1) if half == 0 else (2, 3)
                yps = ps_a.tile([128, 512], DT.float32, tag="acc")
                for i, fc in enumerate(fcs):
                    nc.tensor.matmul(
                        yps[:],
                        wo_sb[:, fc, ec8 * 128:(ec8 + 1) * 128],
                        oT_sb[:, fc, th * 512:(th + 1) * 512],
                        start=(i == 0), stop=(i == 1),
                    )
                ysb = ysp.tile([128, 512], DT.float32, tag="ysb")
                if half == 0 or ec8 % 2 == 1:
                    nc.vector.tensor_copy(ysb[:], yps[:])
                else:
                    nc.scalar.copy(ysb[:], yps[:])
                dst = yA if half == 0 else yB
                nc.sync.dma_start(
                    dst[ec8 * 128:(ec8 + 1) * 128, th * 512:(th + 1) * 512],
                    ysb[:])

            # ---- phase 1: q/k projections for head-pair 0 ----
            for th in range(2):
                proj_fc_th(xq_sb, wq_sb, bq_sb, qT_sb, 0, th)
            for th in range(2):
                proj_fc_th(xk_sb, wk_sb, bk_sb, kT_sb, 0, th)

            # ---- phase 2: attention windows with PE filler work ----
            def mk_proj(x, w, b, d, fc, th):
                return lambda: proj_fc_th(x, w, b, d, fc, th)

            windows = {
                0: [(lambda sc=sc: vproj_sc(sc)) for sc in range(6)]
                   + [mk_proj(xq_sb, wq_sb, bq_sb, qT_sb, 1, th) for th in range(2)]
                   + [mk_proj(xk_sb, wk_sb, bk_sb, kT_sb, 1, th) for th in range(2)],
                1: [(lambda sc=sc: vproj_sc(sc)) for sc in range(6, 8)]
                   + [mk_proj(xq_sb, wq_sb, bq_sb, qT_sb, 2, th) for th in range(2)]
                   + [mk_proj(xk_sb, wk_sb, bk_sb, kT_sb, 2, th) for th in range(2)],
                2: [mk_proj(xq_sb, wq_sb, bq_sb, qT_sb, 3, th) for th in range(2)]
                   + [mk_proj(xk_sb, wk_sb, bk_sb, kT_sb, 3, th) for th in range(2)],
                3: [],
            }

            for c in range(4):
                fillers = windows[c]
                n = len(fillers)
                pvst = {}
                for sc in range(8):
                    qk_sc(c, sc)
                    if c >= 1:
                        pv_slot(c - 1, sc, pvst)
                    if c == 3:
                        # yA: needs pair 0/1 norms, all done by end of w2
                        y_chunk(0, sc, 0)
                        y_chunk(0, sc, 1)
                    for i in range(n * sc // 8, n * (sc + 1) // 8):
                        fillers[i]()

            # ---- phase 3: tail — PV of pair 3 + yB ----
            pvst = {}
            for sc in range(8):
                pv_slot(3, sc, pvst)
                if sc >= 4:
                    # pair-3 th0 norms land at slot 3
                    for e in (2 * (sc - 4), 2 * (sc - 4) + 1):
                        y_chunk(1, e, 0)
            for e in range(8):
                y_chunk(1, e, 1)

    nc.compile()
    return nc


_NC_CACHE = []


def kernel(query, key_, value, edge_bias, attn_mask, key_padding_mask,
           Wq, bq, Wk, bk, Wv, bv, Wo, bo):
    if not _NC_CACHE:
        _NC_CACHE.append(_build_program())
    nc = _NC_CACHE[0]

    scale = np.float32(D ** -0.5)
    q32, k32, v32 = (np.asarray(a, np.float32) for a in (query, key_, value))
    WqT = (np.asarray(Wq, np.float32).T * scale).astype(NP_MMDT)
    WkT = np.asarray(Wk, np.float32).T.astype(NP_MMDT)
    WvT = np.asarray(Wv, np.float32).T.astype(NP_MMDT)
    WoT = np.asarray(Wo, np.float32).T
    bq_s = (np.asarray(bq, np.float32) * scale)
    kpm_add = np.where(np.asarray(key_padding_mask), np.float32(-1e30),
                       np.float32(0.0))  # [B, S]
    mask32 = np.asarray(attn_mask, np.float32)

    def pack_x(xT):
        # [E, T] -> [128, th 2, ec 8, 512] flattened
        return np.ascontiguousarray(
            xT.reshape(8, 128, 2, 512).transpose(1, 2, 0, 3)
            .reshape(128, 8192))

    def pack_w(wT):
        # [E, F] -> [128, fc 4, ec 8, 128] flattened
        return np.ascontiguousarray(
            wT.reshape(8, 128, 4, 128).transpose(1, 2, 0, 3)
            .reshape(128, 4096))

    def pack_wv(wT):
        # [E, F] -> [128, ec 8, 512] flattened
        return np.ascontiguousarray(
            wT.reshape(8, 128, 512).transpose(1, 0, 2).reshape(128, 4096))

    in_maps = []
    for c in range(N_CORES):
        b, g = divmod(c, 2)
        cols = slice(g * F, (g + 1) * F)
        bias_sb = (mask32 + np.asarray(edge_bias[b], np.float32)
                   + kpm_add[b][None, :])  # [T, S]
        in_maps.append({
            "xqT": pack_x(q32[b].T.astype(NP_MMDT)),
            "xkT": pack_x(k32[b].T.astype(NP_MMDT)),
            "xvT": pack_x(v32[b].T.astype(NP_MMDT)),
            "wqT": pack_w(WqT[:, cols]),
            "wkT": pack_w(WkT[:, cols]),
            "wvT": pack_wv(WvT[:, cols]),
            "woT": np.ascontiguousarray(
                WoT[cols, :].astype(NP_MMDT).reshape(4, 128, E)
                .transpose(1, 0, 2).reshape(128, 4 * E)),
            "bq": np.ascontiguousarray(bq_s[cols].reshape(4, 128).T),
            "bk": np.ascontiguousarray(np.asarray(bk, np.float32)[cols]
                                       .reshape(4, 128).T),
            "bv": np.asarray(bv, np.float32)[cols].reshape(1, F),
            "expbT": np.exp(bias_sb.T).astype(NP_MMDT),
        })

    res = run_bass_kernel_spmd(nc, in_maps, list(range(N_CORES)))

    out = np.empty((B, T, E), np.float32)
    bo32 = np.asarray(bo, np.float32)
    for b in range(B):
        r0, r1 = res.results[2 * b], res.results[2 * b + 1]
        acc = r0["yA"] + r0["yB"] + r1["yA"] + r1["yB"]
        out[b] = acc.T + bo32[None, :]
    return out
